# revision 1
# baseline (speedup 1.0000x reference)
"""DIEN-style attention-GRU kernel for 8 trn2 NeuronCores.

Sharding: data-parallel over batch (1024 -> 128 per core), weights
replicated, the time scan stays local per shard. Inputs are fed to the
device pre-transposed ([T, feat, B] per core) so the feature dim sits on
SBUF partitions, which is what the PE contracts over.
"""

import sys

sys.path.insert(0, "/opt/trn_rl_repo")

import numpy as np

import concourse.bacc as bacc
import concourse.mybir as mybir
from concourse.tile import TileContext
from concourse.tile_rust import add_dep_helper
from concourse.bass_utils import run_bass_kernel_spmd

B, T, IN, H = 1024, 200, 128, 128
NCORES = 8
BS = B // NCORES  # 128 batches per core

F32 = mybir.dt.float32
F32R = mybir.dt.float32r
AF = mybir.ActivationFunctionType
ALU = mybir.AluOpType

# dtype knobs (fp32r = fast PE path, fp32 = accurate 4-cyc/row path)
AW_MM_R = False    # attention projection matmul (precision-sensitive)
SCAN_MM_R = True   # scan matmuls (sigmoid/tanh tolerate small error)
USE_AFFINE_ADD = True

GRP = 4            # timesteps per phase-1 matmul group
LOOKAHEAD = 1      # scan x-side pipelining depth


SR = F32R if SCAN_MM_R else F32   # dtype of scan-matmul operand tensors


def _f32(ap):
    return ap.bitcast(F32) if ap.dtype == F32R else ap


def build_nc(t_steps=T, num_devices=NCORES):
    nc = bacc.Bacc("TRN2", target_bir_lowering=False, debug=False,
                   num_devices=num_devices)
    NG = t_steps // GRP
    assert t_steps % GRP == 0

    tgtT = nc.dram_tensor("tgtT", [t_steps, IN, BS], SR, kind="ExternalInput")
    histT = nc.dram_tensor("histT", [t_steps, H, BS], SR, kind="ExternalInput")
    wWT = nc.dram_tensor("wWT", [IN, H], SR, kind="ExternalInput")
    wb_col = nc.dram_tensor("wb_col", [H, 1], F32, kind="ExternalInput")
    WxT4 = nc.dram_tensor("WxT4", [H, 4 * H], SR, kind="ExternalInput")
    WhT = nc.dram_tensor("WhT", [H, 3 * H], SR, kind="ExternalInput")
    bias_row = nc.dram_tensor("bias_row", [1, 4 * H], SR, kind="ExternalInput")
    ln2wh = nc.dram_tensor("ln2wh", [H, H], SR, kind="ExternalInput")
    ln2wt = nc.dram_tensor("ln2wt", [IN, H], SR, kind="ExternalInput")
    ln2b_row = nc.dram_tensor("ln2b_row", [1, H], SR, kind="ExternalInput")
    ident = nc.dram_tensor("ident", [128, 128], F32, kind="ExternalInput")
    ones_r = nc.dram_tensor("ones_r", [1, BS], SR, kind="ExternalInput")
    ones_c = nc.dram_tensor("ones_c", [H, 2], SR, kind="ExternalInput")
    zerosT = nc.dram_tensor("zerosT", [H, BS], SR, kind="ExternalInput")
    out_d = nc.dram_tensor("out", [BS, H], F32, kind="ExternalOutput")

    with TileContext(nc) as tc:
        with (
            tc.tile_pool(name="const", bufs=1) as constp,
            tc.tile_pool(name="hist", bufs=1) as histp,
            tc.tile_pool(name="tgt", bufs=3) as tgtp,
            tc.tile_pool(name="p1", bufs=3) as p1p,
            tc.tile_pool(name="attp", bufs=1) as attp,
            tc.tile_pool(name="scan", bufs=2) as scanp,
            tc.tile_pool(name="state", bufs=2) as statep,
            tc.tile_pool(name="awps", bufs=2, space="PSUM") as awps,
            tc.tile_pool(name="lgps", bufs=1, space="PSUM") as lgps,
            tc.tile_pool(name="xqps", bufs=3, space="PSUM") as xqps,
            tc.tile_pool(name="trps", bufs=1, space="PSUM") as trps,
        ):
            # ---- constants / weights into SBUF ----
            def cload(dram, shape, dt=F32):
                t = constp.tile(shape, dt, tag=dram.name)
                nc.sync.dma_start(t[:], dram[:, :])
                return t

            wWT_s = cload(wWT, [IN, H], SR)
            wb_s = cload(wb_col, [H, 1])
            WxT4_s = cload(WxT4, [H, 4 * H], SR)
            WhT_s = cload(WhT, [H, 3 * H], SR)
            bias_s = cload(bias_row, [1, 4 * H], SR)
            ln2wh_s = cload(ln2wh, [H, H], SR)
            ln2wt_s = cload(ln2wt, [IN, H], SR)
            ln2b_s = cload(ln2b_row, [1, H], SR)
            ident_s = cload(ident, [128, 128])
            ones_row = cload(ones_r, [1, BS], SR)
            ones_col = cload(ones_c, [H, 2], SR)

            hist_all = histp.tile([128, t_steps, BS], SR, tag="hist_all")
            warm = awps.tile([BS, 4 * H], F32, tag="aw")
            for _ in range(8):
                nc.tensor.matmul(warm[:], WxT4_s[:, 0:H], WxT4_s[:],
                                 start=True, stop=True)
            logits_ps = lgps.tile([BS, t_steps, 2], F32, tag="logits")

            # ---- phase 1: attention logits ----
            aw_tiles = {}

            def p1_load_mm(g):
                t0 = g * GRP
                tg = tgtp.tile([128, GRP, BS], SR, tag="tgt")
                nc.sync.dma_start(
                    tg[:], tgtT[t0:t0 + GRP, :, :].rearrange("t i b -> i t b"))
                nc.sync.dma_start(
                    hist_all[:, t0:t0 + GRP, :],
                    histT[t0:t0 + GRP, :, :].rearrange("t h b -> h t b"))
                awt = awps.tile([H, GRP * BS], F32, tag="aw")
                nc.tensor.matmul(awt[:],
                                 wWT_s[:],
                                 tg[:].rearrange("i t b -> i (t b)"),
                                 start=True, stop=True)
                aw_tiles[g] = awt

            def p1_reduce(g):
                t0 = g * GRP
                awt = aw_tiles.pop(g)
                awb = p1p.tile([H, GRP * BS], F32, tag="awb")
                nc.scalar.activation(awb[:], awt[:], AF.Identity, bias=wb_s[:])
                prod = p1p.tile([H, GRP * BS], SR, tag="prod")
                nc.vector.tensor_tensor(
                    prod[:], awb[:],
                    _f32(hist_all[:, t0:t0 + GRP, :].rearrange("h t b -> h (t b)")),
                    ALU.mult)
                for j in range(GRP):
                    nc.tensor.matmul(
                        logits_ps[:, t0 + j, :],
                        prod[:, j * BS:(j + 1) * BS],
                        ones_col[:],
                        start=True, stop=True)

            P1LA = 2
            for g in range(-P1LA, NG):
                if g + P1LA < NG:
                    p1_load_mm(g + P1LA)
                if g >= 0:
                    p1_reduce(g)

            # ---- softmax over time (free dim), natural [B, T] layout ----
            mx = attp.tile([BS, 1], F32, tag="mx")
            nc.vector.tensor_reduce(mx[:], logits_ps[:, :, 0], mybir.AxisListType.X,
                                    ALU.max)
            negmx = attp.tile([BS, 1], F32, tag="negmx")
            nc.vector.tensor_scalar_mul(negmx[:], mx[:], -1.0)
            exps = attp.tile([BS, t_steps], F32, tag="exps")
            nc.scalar.activation(exps[:], logits_ps[:, :, 0], AF.Exp, bias=negmx[:])
            ssum = attp.tile([BS, 1], F32, tag="ssum")
            nc.vector.tensor_reduce(ssum[:], exps[:], mybir.AxisListType.X,
                                    ALU.add)
            rsum = attp.tile([BS, 1], F32, tag="rsum")
            nc.vector.reciprocal(rsum[:], ssum[:])
            att = attp.tile([BS, t_steps], F32, tag="att")
            nc.vector.tensor_scalar_mul(att[:], exps[:], rsum[:])
            nega = attp.tile([BS, t_steps], F32, tag="nega")
            nc.vector.tensor_scalar_mul(nega[:], att[:], -1.0)

            # ---- phase 2: the scan ----
            # psum bank layout per step: [pre_u | pre_r | s | q] (4 x 128)
            #   pre_u = xu + bu + mmu ; pre_r = xr + br + mmr
            #   s = xg + xg_b + hg_b + mmg ; q = xg + xg_b
            h_t = statep.tile([BS, H], F32, tag="h")
            hT_t = statep.tile([H, BS], SR, tag="hT")
            nc.vector.memset(h_t[:], 0.0)
            nc.sync.dma_start(hT_t[:], zerosT[:, :])

            xq_tiles = {}

            def fill(t):
                # bank layout [pre_u | pre_r | m | q]:
                #   m-cols get hg_b (bias) + mmg (mm_h); zeros from mm_x
                #   q-cols get xg_b (bias) + xg (mm_x)
                xq = xqps.tile([BS, 4 * H], F32, tag="xq")
                nc.tensor.matmul(xq[:], ones_row[:],
                                 bias_s[:],
                                 start=True, stop=False)
                nc.tensor.matmul(xq[:],
                                 hist_all[:, t, :],
                                 WxT4_s[:],
                                 start=False, stop=True)
                q = scanp.tile([BS, H], F32, tag="q")
                nc.scalar.activation(q[:], xq[:, 3 * H:4 * H], AF.Copy)
                xq_tiles[t] = (xq, q)

            def consume(t, h_cur, hT_cur):
                xq, q = xq_tiles.pop(t)
                nc.tensor.matmul(xq[:, 0:3 * H],
                                 hT_cur[:],
                                 WhT_s[:],
                                 start=False, stop=True, skip_group_check=True)
                sru = scanp.tile([BS, 2 * H], F32, tag="sru")
                nc.scalar.activation(sru[:], xq[:, 0:2 * H], AF.Sigmoid)
                su = sru[:, 0:H]
                gin = scanp.tile([BS, H], F32, tag="gin")
                bi_gin = nc.vector.tensor_tensor(gin[:], sru[:, H:2 * H],
                                                 xq[:, 2 * H:3 * H], ALU.mult)
                gpre = scanp.tile([BS, H], F32, tag="gpre")
                bi_gpre = nc.vector.tensor_tensor(gpre[:], gin[:], q[:], ALU.add)
                g_ = scanp.tile([BS, H], F32, tag="g")
                nc.scalar.activation(g_[:], gpre[:], AF.Tanh)
                p_ = scanp.tile([BS, H], F32, tag="p")
                bi_p = nc.vector.tensor_tensor(p_[:], su, g_[:], ALU.mult)
                um = scanp.tile([BS, H], F32, tag="um")
                bi_um = nc.vector.tensor_scalar(um[:], su, nega[:, t:t + 1],
                                                1.0, ALU.mult, ALU.add)
                t2 = scanp.tile([BS, H], F32, tag="t2")
                nc.vector.tensor_tensor(t2[:], um[:], h_cur[:], ALU.mult)
                h_new = statep.tile([BS, H], F32, tag="h")
                nc.vector.affine_then_add(h_new[:], p_[:], t2[:],
                                          att[:, t:t + 1], 0.0)
                # keep the r-chain ahead of the off-chain um on the in-order DVE
                add_dep_helper(bi_um.ins, bi_gpre.ins, sync=False,
                               reason="scan: um after gpre (DVE order)")
                trp = trps.tile([H, BS], F32, tag="tr")
                nc.tensor.transpose(trp[:], h_new[:], ident_s[:])
                hT_new = statep.tile([H, BS], SR, tag="hT")
                nc.vector.tensor_copy(hT_new[:], trp[:])
                return h_new, hT_new

            for t in range(-LOOKAHEAD, t_steps):
                if t + LOOKAHEAD < t_steps:
                    fill(t + LOOKAHEAD)
                if t >= 0:
                    h_t, hT_t = consume(t, h_t, hT_t)

            # ---- phase 3: out = [h, targets[:,0]] @ ln2_w.T + ln2_b ----
            t0T = scanp.tile([IN, BS], SR, tag="t0T")
            nc.sync.dma_start(t0T[:], tgtT[0, :, :])
            ops = trps.tile([BS, H], F32, tag="tr")
            nc.tensor.matmul(ops[:], ones_row[:], ln2b_s[:], start=True,
                             stop=False)
            nc.tensor.matmul(ops[:], hT_t[:], ln2wh_s[:], start=False,
                             stop=False)
            nc.tensor.matmul(ops[:], t0T[:], ln2wt_s[:], start=False, stop=True)
            out_s = scanp.tile([BS, H], F32, tag="out_s")
            nc.vector.tensor_copy(out_s[:], ops[:])
            nc.sync.dma_start(out_d[:, :], out_s[:])

    nc.compile()
    return nc


def make_weight_feeds(inputs, t_steps=T):
    f32 = np.float32
    xu_w, xu_b = inputs["xu_w"], inputs["xu_b"]
    hu_w, hu_b = inputs["hu_w"], inputs["hu_b"]
    xr_w, xr_b = inputs["xr_w"], inputs["xr_b"]
    hr_w, hr_b = inputs["hr_w"], inputs["hr_b"]
    xg_w, xg_b = inputs["xg_w"], inputs["xg_b"]
    hg_w, hg_b = inputs["hg_w"], inputs["hg_b"]
    ln2_w, ln2_b = inputs["ln2_w"], inputs["ln2_b"]
    feeds = {
        "wWT": np.ascontiguousarray(np.asarray(inputs["W_w"]).T, dtype=f32),
        "wb_col": np.asarray(inputs["W_b"], dtype=f32).reshape(H, 1).copy(),
        "WxT4": np.ascontiguousarray(
            np.concatenate([np.asarray(xu_w).T, np.asarray(xr_w).T,
                            np.zeros((H, H)), np.asarray(xg_w).T],
                           axis=1), dtype=f32),
        "WhT": np.ascontiguousarray(
            np.concatenate([np.asarray(w).T for w in (hu_w, hr_w, hg_w)],
                           axis=1), dtype=f32),
        "bias_row": np.concatenate([
            np.asarray(xu_b) + np.asarray(hu_b),
            np.asarray(xr_b) + np.asarray(hr_b),
            np.asarray(hg_b),
            np.asarray(xg_b)]).astype(f32).reshape(1, 4 * H).copy(),
        "ln2wh": np.ascontiguousarray(np.asarray(ln2_w)[:, :H].T, dtype=f32),
        "ln2wt": np.ascontiguousarray(np.asarray(ln2_w)[:, H:].T, dtype=f32),
        "ln2b_row": np.asarray(ln2_b, dtype=f32).reshape(1, H).copy(),
        "ident": np.eye(128, dtype=f32),
        "ones_r": np.ones((1, BS), dtype=f32),
        "ones_c": np.ones((H, 2), dtype=f32),
        "zerosT": np.zeros((H, BS), dtype=f32),
    }
    return feeds


def make_core_feeds(inputs, core, t_steps=T):
    sl = slice(core * BS, (core + 1) * BS)
    tgt = np.asarray(inputs["targets"])[sl, :t_steps]
    hist = np.asarray(inputs["history_states"])[sl, :t_steps]
    return {
        "tgtT": np.ascontiguousarray(tgt.transpose(1, 2, 0), dtype=np.float32),
        "histT": np.ascontiguousarray(hist.transpose(1, 2, 0), dtype=np.float32),
    }


_nc_cache = {}


def _get_nc(t_steps=T):
    if t_steps not in _nc_cache:
        _nc_cache[t_steps] = build_nc(t_steps)
    return _nc_cache[t_steps]


def kernel(**inputs):
    nc = _get_nc(T)
    wf = make_weight_feeds(inputs)
    in_maps = [{**make_core_feeds(inputs, c), **wf} for c in range(NCORES)]
    res = run_bass_kernel_spmd(nc, in_maps, list(range(NCORES)))
    out = np.concatenate([res.results[c]["out"] for c in range(NCORES)], axis=0)
    return out.astype(np.float32)



# revision 4
# speedup vs baseline: 1.4555x; 1.4555x over previous
"""DIEN-style attention-GRU kernel for 8 trn2 NeuronCores.

Sharding: data-parallel over batch (1024 -> 128 per core), weights
replicated, the time scan stays local per shard.

Design (v2): everything runs in the transposed layout [feat, batch] so the
recurrent state h^T is the *moving* matmul operand (SBUF bf16) and the small
[128,128] weights are the stationary operands.  This removes the per-step
PE-transpose + PSUM->SBUF cast from the serial chain.  All tensors are bf16
(fp32 PSUM accumulation); biases are folded into ACT bias / fused
scalar_tensor_tensor ops.  Attention coefficients are broadcast across
partitions with tiny PE outer-products, one 4-step chunk ahead of the scan.
"""

import sys

sys.path.insert(0, "/opt/trn_rl_repo")

import numpy as np
import ml_dtypes

import concourse.bacc as bacc
import concourse.mybir as mybir
from concourse.tile import TileContext
from concourse.bass_utils import run_bass_kernel_spmd

B, T, IN, H = 1024, 200, 128, 128
NCORES = 8
BS = B // NCORES  # 128 batches per core

F32 = mybir.dt.float32
BF16 = mybir.dt.bfloat16
AF = mybir.ActivationFunctionType
ALU = mybir.AluOpType

PG = 8    # phase-1 timesteps per chunk (2 PSUM banks of aw per chunk)
P1LA = 2  # phase-1 chunk lookahead
XLA = 2   # scan x-side lookahead (steps)


def build_nc(t_steps=T, num_devices=NCORES):
    nc = bacc.Bacc("TRN2", target_bir_lowering=False, debug=False,
                   num_devices=num_devices)
    NPG = t_steps // PG
    assert t_steps % PG == 0 and t_steps % 4 == 0

    tgtT = nc.dram_tensor("tgtT", [IN, t_steps, BS], BF16, kind="ExternalInput")
    histT = nc.dram_tensor("histT", [H, t_steps, BS], BF16, kind="ExternalInput")
    wWT = nc.dram_tensor("wWT", [IN, H], BF16, kind="ExternalInput")
    wb_col = nc.dram_tensor("wb_col", [H, 1], F32, kind="ExternalInput")
    WhuT = nc.dram_tensor("WhuT", [H, H], BF16, kind="ExternalInput")
    WhrT = nc.dram_tensor("WhrT", [H, H], BF16, kind="ExternalInput")
    WhgT = nc.dram_tensor("WhgT", [H, H], BF16, kind="ExternalInput")
    WxuT = nc.dram_tensor("WxuT", [H, H], BF16, kind="ExternalInput")
    WxrT = nc.dram_tensor("WxrT", [H, H], BF16, kind="ExternalInput")
    WxgT = nc.dram_tensor("WxgT", [H, H], BF16, kind="ExternalInput")
    bu_col = nc.dram_tensor("bu_col", [H, 1], F32, kind="ExternalInput")
    br_col = nc.dram_tensor("br_col", [H, 1], F32, kind="ExternalInput")
    bg_col = nc.dram_tensor("bg_col", [H, 1], F32, kind="ExternalInput")
    bq_col = nc.dram_tensor("bq_col", [H, 1], F32, kind="ExternalInput")
    ln2wh = nc.dram_tensor("ln2wh", [H, H], BF16, kind="ExternalInput")
    ln2wt = nc.dram_tensor("ln2wt", [IN, H], BF16, kind="ExternalInput")
    ln2b_row = nc.dram_tensor("ln2b_row", [1, H], BF16, kind="ExternalInput")
    ident = nc.dram_tensor("ident", [128, 128], BF16, kind="ExternalInput")
    ones_row = nc.dram_tensor("ones_row", [1, 128], BF16, kind="ExternalInput")
    ones_col = nc.dram_tensor("ones_col", [H, 1], BF16, kind="ExternalInput")
    out_d = nc.dram_tensor("out", [BS, H], F32, kind="ExternalOutput")

    with TileContext(nc) as tc:
        with (
            tc.tile_pool(name="const", bufs=1) as constp,
            tc.tile_pool(name="hist", bufs=1) as histp,
            tc.tile_pool(name="tgt", bufs=3) as tgtp,
            tc.tile_pool(name="p1sb", bufs=3) as p1sb,
            tc.tile_pool(name="att", bufs=1) as attp,
            tc.tile_pool(name="scan", bufs=3) as scanp,
            tc.tile_pool(name="state", bufs=3) as statep,
        ):
            # ---- constants / weights into SBUF ----
            def cload(dram, shape, dt):
                t = constp.tile(shape, dt, tag=dram.name)
                nc.sync.dma_start(t[:], dram[:, :])
                return t

            wWT_s = cload(wWT, [IN, H], BF16)
            wb_s = cload(wb_col, [H, 1], F32)
            WhuT_s = cload(WhuT, [H, H], BF16)
            WhrT_s = cload(WhrT, [H, H], BF16)
            WhgT_s = cload(WhgT, [H, H], BF16)
            WxuT_s = cload(WxuT, [H, H], BF16)
            WxrT_s = cload(WxrT, [H, H], BF16)
            WxgT_s = cload(WxgT, [H, H], BF16)
            bu_s = cload(bu_col, [H, 1], F32)
            br_s = cload(br_col, [H, 1], F32)
            bg_s = cload(bg_col, [H, 1], F32)
            bq_s = cload(bq_col, [H, 1], F32)
            ln2wh_s = cload(ln2wh, [H, H], BF16)
            ln2wt_s = cload(ln2wt, [IN, H], BF16)
            ln2b_s = cload(ln2b_row, [1, H], BF16)
            ident_s = cload(ident, [128, 128], BF16)
            ones_row_s = cload(ones_row, [1, 128], BF16)
            ones_col_s = cload(ones_col, [H, 1], BF16)

            hist_all = histp.tile([128, t_steps, BS], BF16, tag="hist_all")
            # att transposed: row (t % 100), half (t // 100)
            attT = attp.tile([100, 2, BS], BF16, tag="attT")

            # ================= phase 1: attention =================
            with (
                tc.tile_pool(name="awps", bufs=2, space="PSUM") as awps,
                tc.tile_pool(name="lgps", bufs=1, space="PSUM") as lgps,
            ):
                logits_ps = lgps.tile([BS, t_steps], F32, tag="logits")
                aw_tiles = {}

                def p1_load_mm(g):
                    t0 = g * PG
                    tg = tgtp.tile([128, PG, BS], BF16, tag="tgt")
                    nc.sync.dma_start(tg[:], tgtT[:, t0:t0 + PG, :])
                    nc.sync.dma_start(hist_all[:, t0:t0 + PG, :],
                                      histT[:, t0:t0 + PG, :])
                    awt = awps.tile([H, PG, BS], F32, tag="aw")
                    hpg = PG // 2
                    nc.tensor.matmul(
                        awt[:, 0:hpg, :].rearrange("i t b -> i (t b)"),
                        wWT_s[:],
                        tg[:, 0:hpg, :].rearrange("i t b -> i (t b)"),
                        start=True, stop=True)
                    nc.tensor.matmul(
                        awt[:, hpg:PG, :].rearrange("i t b -> i (t b)"),
                        wWT_s[:],
                        tg[:, hpg:PG, :].rearrange("i t b -> i (t b)"),
                        start=True, stop=True)
                    aw_tiles[g] = awt

                def p1_reduce(g):
                    t0 = g * PG
                    awt = aw_tiles.pop(g)
                    # prod = (aw + W_b) * hist   (W_b fused per-partition)
                    prod = p1sb.tile([H, PG, BS], BF16, tag="prod")
                    nc.vector.scalar_tensor_tensor(
                        prod[:].rearrange("h t b -> h (t b)"),
                        awt[:].rearrange("h t b -> h (t b)"),
                        wb_s[:],
                        hist_all[:, t0:t0 + PG, :].rearrange("h t b -> h (t b)"),
                        ALU.add, ALU.mult)
                    # logits[:, t] = ones^T . prod_t  (partition reduce on PE)
                    for j in range(PG):
                        nc.tensor.matmul(
                            logits_ps[:, t0 + j:t0 + j + 1],
                            prod[:, j, :],
                            ones_col_s[:],
                            start=True, stop=True)

                for g in range(-P1LA, NPG):
                    if g + P1LA < NPG:
                        p1_load_mm(g + P1LA)
                    if g >= 0:
                        p1_reduce(g)

                # ---- softmax over time (free dim) ----
                mx = attp.tile([BS, 1], F32, tag="mx")
                nc.vector.tensor_reduce(mx[:], logits_ps[:], mybir.AxisListType.X,
                                        ALU.max)
                negmx = attp.tile([BS, 1], F32, tag="negmx")
                nc.vector.tensor_scalar_mul(negmx[:], mx[:], -1.0)
                exps = attp.tile([BS, t_steps], F32, tag="exps")
                nc.scalar.activation(exps[:], logits_ps[:], AF.Exp, bias=negmx[:])
                ssum = attp.tile([BS, 1], F32, tag="ssum")
                nc.vector.tensor_reduce(ssum[:], exps[:], mybir.AxisListType.X,
                                        ALU.add)
                rsum = attp.tile([BS, 1], F32, tag="rsum")
                nc.vector.reciprocal(rsum[:], ssum[:])
                att = attp.tile([BS, t_steps], BF16, tag="attn")
                nc.vector.tensor_scalar_mul(att[:], exps[:], rsum[:])
                # transpose att -> attT rows (PE transpose, two halves)
                for hf in range(2):
                    tps = awps.tile([100, BS], BF16, tag="aw")
                    nc.tensor.transpose(tps[:], att[:, hf * 100:(hf + 1) * 100],
                                        ident_s[:])
                    nc.vector.tensor_copy(attT[:, hf, :], tps[:])

            # ================= phase 2: the scan =================
            with (
                tc.tile_pool(name="rmps", bufs=3, space="PSUM") as rmps,
                tc.tile_pool(name="uqps", bufs=3, space="PSUM") as uqps,
                tc.tile_pool(name="abps", bufs=2, space="PSUM") as abps,
            ):
                h_t = statep.tile([H, BS], BF16, tag="h")
                nc.vector.memset(h_t[:], 0.0)

                rm_tiles = {}
                uq_tiles = {}
                abc_tiles = {}

                def abc_fill(c):
                    # broadcast att[:, 4c:4c+4] across partitions: gather the
                    # four attT rows onto partition 0, then one PE
                    # outer-product  ab = ones(128) x att_row4
                    t0 = c * 4
                    row4 = scanp.tile([1, 4, BS], BF16, tag="arow")
                    nc.sync.dma_start(
                        row4[:], attT[t0 % 100:t0 % 100 + 4, t0 // 100, :])
                    ab = abps.tile([128, 4, BS], F32, tag="abc")
                    nc.tensor.matmul(
                        ab[:].rearrange("p t b -> p (t b)"),
                        ones_row_s[:],
                        row4[:].rearrange("p t b -> p (t b)"),
                        start=True, stop=True)
                    abc_tiles[c] = ab

                def x_fill(t):
                    # x-side projections for step t into the step's PSUM banks
                    # rm bank: [r | m],  uq bank: [u | q]
                    ht = hist_all[:, t, :]
                    rmt = rmps.tile([H, 2, BS], F32, tag="rm")
                    uqt = uqps.tile([H, 2, BS], F32, tag="uq")
                    nc.tensor.matmul(rmt[:, 0, :], WxrT_s[:], ht,
                                     start=True, stop=False)
                    nc.tensor.matmul(uqt[:, 0, :], WxuT_s[:], ht,
                                     start=True, stop=False)
                    nc.tensor.matmul(uqt[:, 1, :], WxgT_s[:], ht,
                                     start=False, stop=False,
                                     skip_group_check=True)
                    rm_tiles[t] = rmt
                    uq_tiles[t] = uqt

                def consume(t, h_cur):
                    rmt = rm_tiles.pop(t)
                    uqt = uq_tiles.pop(t)
                    ab = abc_tiles[t // 4]
                    # h-side matmuls: stationary = weights, moving = h^T
                    nc.tensor.matmul(rmt[:, 0, :], WhrT_s[:], h_cur[:],
                                     start=False, stop=False,
                                     skip_group_check=True)
                    nc.tensor.matmul(rmt[:, 1, :], WhgT_s[:], h_cur[:],
                                     start=False, stop=True,
                                     skip_group_check=True)
                    nc.tensor.matmul(uqt[:, 0, :], WhuT_s[:], h_cur[:],
                                     start=False, stop=True,
                                     skip_group_check=True)
                    r = scanp.tile([H, BS], BF16, tag="r")
                    nc.scalar.activation(r[:], rmt[:, 0, :], AF.Sigmoid,
                                         bias=br_s[:])
                    u = scanp.tile([H, BS], BF16, tag="u")
                    nc.scalar.activation(u[:], uqt[:, 0, :], AF.Sigmoid,
                                         bias=bu_s[:])
                    # rm = (mg + bg) * r
                    rm = scanp.tile([H, BS], BF16, tag="rmv")
                    nc.vector.scalar_tensor_tensor(rm[:], rmt[:, 1, :], bg_s[:],
                                                   r[:], ALU.add, ALU.mult)
                    # gpre = (xq + bq) + rm
                    gpre = scanp.tile([H, BS], BF16, tag="gpre")
                    nc.vector.scalar_tensor_tensor(gpre[:], uqt[:, 1, :], bq_s[:],
                                                   rm[:], ALU.add, ALU.add)
                    g_ = scanp.tile([H, BS], BF16, tag="g")
                    nc.scalar.activation(g_[:], gpre[:], AF.Tanh)
                    # v = a_t * u ; um = 1 - v ; t2 = um * h ; s1 = g * v
                    v = scanp.tile([H, BS], BF16, tag="v")
                    nc.vector.tensor_tensor(v[:], u[:], ab[:, t % 4, :], ALU.mult)
                    um = scanp.tile([H, BS], BF16, tag="um")
                    nc.vector.tensor_scalar(um[:], v[:], -1.0, 1.0,
                                            ALU.mult, ALU.add)
                    t2 = scanp.tile([H, BS], BF16, tag="t2")
                    nc.vector.tensor_tensor(t2[:], um[:], h_cur[:], ALU.mult)
                    s1 = scanp.tile([H, BS], BF16, tag="s1")
                    nc.vector.tensor_tensor(s1[:], g_[:], v[:], ALU.mult)
                    h_new = statep.tile([H, BS], BF16, tag="h")
                    nc.vector.tensor_tensor(h_new[:], s1[:], t2[:], ALU.add)
                    return h_new

                abc_fill(0)
                for t in range(-XLA, t_steps):
                    tf = t + XLA
                    if tf < t_steps:
                        if tf % 4 == 0 and tf > 0:
                            abc_fill(tf // 4)
                        x_fill(tf)
                    if t >= 0:
                        h_t = consume(t, h_t)

                # ---- phase 3: out = [h, targets[:,0]] @ ln2_w.T + ln2_b ----
                t0T = scanp.tile([IN, BS], BF16, tag="t0T")
                nc.sync.dma_start(t0T[:], tgtT[:, 0, :])
                ops = rmps.tile([BS, H], F32, tag="rm")
                nc.tensor.matmul(ops[:], ones_row_s[:], ln2b_s[:],
                                 start=True, stop=False)
                nc.tensor.matmul(ops[:], h_t[:], ln2wh_s[:],
                                 start=False, stop=False, skip_group_check=True)
                nc.tensor.matmul(ops[:], t0T[:], ln2wt_s[:],
                                 start=False, stop=True, skip_group_check=True)
                out_s = scanp.tile([BS, H], F32, tag="out_s")
                nc.vector.tensor_copy(out_s[:], ops[:])
                nc.sync.dma_start(out_d[:, :], out_s[:])

    nc.compile()
    return nc


def make_weight_feeds(inputs, t_steps=T):
    f32 = np.float32
    bf16 = ml_dtypes.bfloat16

    def tb(x):  # transpose to [in, out] and cast bf16
        return np.ascontiguousarray(np.asarray(x, dtype=f32).T).astype(bf16)

    feeds = {
        "wWT": tb(inputs["W_w"]),
        "wb_col": np.asarray(inputs["W_b"], dtype=f32).reshape(H, 1).copy(),
        "WhuT": tb(inputs["hu_w"]),
        "WhrT": tb(inputs["hr_w"]),
        "WhgT": tb(inputs["hg_w"]),
        "WxuT": tb(inputs["xu_w"]),
        "WxrT": tb(inputs["xr_w"]),
        "WxgT": tb(inputs["xg_w"]),
        "bu_col": (np.asarray(inputs["xu_b"], f32)
                   + np.asarray(inputs["hu_b"], f32)).reshape(H, 1).copy(),
        "br_col": (np.asarray(inputs["xr_b"], f32)
                   + np.asarray(inputs["hr_b"], f32)).reshape(H, 1).copy(),
        "bg_col": np.asarray(inputs["hg_b"], f32).reshape(H, 1).copy(),
        "bq_col": np.asarray(inputs["xg_b"], f32).reshape(H, 1).copy(),
        "ln2wh": np.ascontiguousarray(
            np.asarray(inputs["ln2_w"], f32)[:, :H].T).astype(bf16),
        "ln2wt": np.ascontiguousarray(
            np.asarray(inputs["ln2_w"], f32)[:, H:].T).astype(bf16),
        "ln2b_row": np.asarray(inputs["ln2_b"], f32).reshape(1, H).astype(bf16),
        "ident": np.eye(128, dtype=f32).astype(bf16),
        "ones_row": np.ones((1, 128), dtype=f32).astype(bf16),
        "ones_col": np.ones((H, 1), dtype=f32).astype(bf16),
    }
    return feeds


def make_core_feeds(inputs, core, t_steps=T):
    bf16 = ml_dtypes.bfloat16
    sl = slice(core * BS, (core + 1) * BS)
    tgt = np.asarray(inputs["targets"])[sl, :t_steps]
    hist = np.asarray(inputs["history_states"])[sl, :t_steps]
    return {
        # [BS, T, F] -> [F, T, BS]
        "tgtT": np.ascontiguousarray(tgt.transpose(2, 1, 0)).astype(bf16),
        "histT": np.ascontiguousarray(hist.transpose(2, 1, 0)).astype(bf16),
    }


_nc_cache = {}


def _get_nc(t_steps=T):
    if t_steps not in _nc_cache:
        _nc_cache[t_steps] = build_nc(t_steps)
    return _nc_cache[t_steps]


def kernel(**inputs):
    nc = _get_nc(T)
    wf = make_weight_feeds(inputs)
    in_maps = [{**make_core_feeds(inputs, c), **wf} for c in range(NCORES)]
    res = run_bass_kernel_spmd(nc, in_maps, list(range(NCORES)))
    out = np.concatenate([res.results[c]["out"] for c in range(NCORES)], axis=0)
    return out.astype(np.float32)


# revision 5
# speedup vs baseline: 1.4847x; 1.0201x over previous
"""DIEN-style attention-GRU kernel for 8 trn2 NeuronCores.

Sharding: data-parallel over batch (1024 -> 128 per core), weights
replicated, the time scan stays local per shard.

v3: transposed layout [feat, batch] everywhere, bf16 compute with fp32 PSUM
accumulation.  The recurrent update h' = s1 + t2 is *not* materialized on
the critical path: by linearity Wh.h' = Wh.s1 + Wh.t2, so the next step's
r-gate matmul accumulates the t2-stream early (during tanh) and only the
s1-stream (128 cols) remains serial.  Biases ride in ACT bias slots, a PE
outer-product (bg), and fused scalar_tensor_tensor ops.  Attention weights
are broadcast across partitions with one PE outer-product per 4-step chunk.
"""

import sys

sys.path.insert(0, "/opt/trn_rl_repo")

import numpy as np
import ml_dtypes

import concourse.bacc as bacc
import concourse.mybir as mybir
from concourse.tile import TileContext
from concourse.tile_rust import add_dep_helper
from concourse.bass_utils import run_bass_kernel_spmd

B, T, IN, H = 1024, 200, 128, 128
NCORES = 8
BS = B // NCORES  # 128 batches per core

F32 = mybir.dt.float32
BF16 = mybir.dt.bfloat16
AF = mybir.ActivationFunctionType
ALU = mybir.AluOpType

PG = 8    # phase-1 timesteps per chunk (2 PSUM banks of aw per chunk)
XLA = 2   # scan x-side lookahead (steps)
NDMA = 4  # big input DMAs per tensor

# wcat block indices
W_AW, W_HU, W_HR, W_HG, W_XU, W_XR, W_XG, W_L2H, W_L2T, W_ID, W_ONE = range(11)
NBLK = 11


def build_nc(t_steps=T, num_devices=NCORES):
    nc = bacc.Bacc("TRN2", target_bir_lowering=False, debug=False,
                   num_devices=num_devices)
    NPG = t_steps // PG
    NC4 = t_steps // 4
    assert t_steps % PG == 0 and t_steps % 4 == 0

    tgtT = nc.dram_tensor("tgtT", [IN, t_steps, BS], BF16, kind="ExternalInput")
    histT = nc.dram_tensor("histT", [H, t_steps, BS], BF16,
                           kind="ExternalInput")
    wcat = nc.dram_tensor("wcat", [128, NBLK * 128], BF16, kind="ExternalInput")
    bcols = nc.dram_tensor("bcols", [H, 4], F32, kind="ExternalInput")
    brows = nc.dram_tensor("brows", [1, 2 * 128], BF16, kind="ExternalInput")
    out_d = nc.dram_tensor("out", [BS, H], F32, kind="ExternalOutput")

    with TileContext(nc) as tc:
        with (
            tc.tile_pool(name="const", bufs=1) as constp,
            tc.tile_pool(name="big", bufs=1) as bigp,
            tc.tile_pool(name="p1sb", bufs=3) as p1sb,
            tc.tile_pool(name="att", bufs=1) as attp,
            tc.tile_pool(name="scan", bufs=3) as scanp,
            tc.tile_pool(name="state", bufs=3) as statep,
        ):
            # ---- constants (3 DMAs) ----
            wcat_s = constp.tile([128, NBLK * 128], BF16, tag="wcat")
            nc.sync.dma_start(wcat_s[:], wcat[:, :])
            bcols_s = constp.tile([H, 4], F32, tag="bcols")
            nc.sync.dma_start(bcols_s[:], bcols[:, :])
            brows_s = constp.tile([1, 2 * 128], BF16, tag="brows")
            nc.sync.dma_start(brows_s[:], brows[:, :])

            def blk(i):
                return wcat_s[:, i * 128:(i + 1) * 128]

            wWT_s = blk(W_AW)
            WhuT_s, WhrT_s, WhgT_s = blk(W_HU), blk(W_HR), blk(W_HG)
            WxuT_s, WxrT_s, WxgT_s = blk(W_XU), blk(W_XR), blk(W_XG)
            ln2wh_s, ln2wt_s, ident_s = blk(W_L2H), blk(W_L2T), blk(W_ID)
            ones_row_s = wcat_s[0:1, W_ONE * 128:(W_ONE + 1) * 128]
            ones_col_s = wcat_s[:, W_ONE * 128:W_ONE * 128 + 1]
            wb_s = bcols_s[:, 0:1]
            bu_s = bcols_s[:, 1:2]
            br_s = bcols_s[:, 2:3]
            bq_s = bcols_s[:, 3:4]
            ln2b_s = brows_s[:, 0:H]
            bg_row_s = brows_s[:, 128:128 + H]

            # ---- big input DMAs (all upfront, 4 slices each) ----
            tgt_all = bigp.tile([128, t_steps, BS], BF16, tag="tgt_all")
            hist_all = bigp.tile([128, t_steps, BS], BF16, tag="hist_all")
            TSL = t_steps // NDMA
            for d in range(NDMA):
                sl = slice(d * TSL, (d + 1) * TSL)
                nc.sync.dma_start(tgt_all[:, sl, :], tgtT[:, sl, :])
                nc.sync.dma_start(hist_all[:, sl, :], histT[:, sl, :])

            # warm the ACT exp table while DMA streams
            dummy = attp.tile([1, 1], F32, tag="dummy")
            nc.vector.memset(dummy[:], 0.0)
            nc.scalar.activation(dummy[:], dummy[:], AF.Exp)

            attT = attp.tile([100, 2, BS], BF16, tag="attT")

            # ================= phase 1: attention =================
            with (
                tc.tile_pool(name="awps", bufs=2, space="PSUM") as awps,
                tc.tile_pool(name="lgps", bufs=1, space="PSUM") as lgps,
            ):
                logits_ps = lgps.tile([BS, t_steps], F32, tag="logits")
                for g in range(NPG):
                    t0 = g * PG
                    awt = awps.tile([H, PG, BS], F32, tag="aw")
                    hpg = PG // 2
                    for hh in range(2):
                        s2 = slice(t0 + hh * hpg, t0 + (hh + 1) * hpg)
                        nc.tensor.matmul(
                            awt[:, hh * hpg:(hh + 1) * hpg, :].rearrange(
                                "i t b -> i (t b)"),
                            wWT_s,
                            tgt_all[:, s2, :].rearrange("i t b -> i (t b)"),
                            start=True, stop=True)
                    # prod = (aw + W_b) * hist   (W_b fused per-partition)
                    prod = p1sb.tile([H, PG, BS], BF16, tag="prod")
                    nc.vector.scalar_tensor_tensor(
                        prod[:].rearrange("h t b -> h (t b)"),
                        awt[:].rearrange("h t b -> h (t b)"),
                        wb_s,
                        hist_all[:, t0:t0 + PG, :].rearrange("h t b -> h (t b)"),
                        ALU.add, ALU.mult)
                    # logits[:, t] = ones^T . prod_t  (partition reduce on PE)
                    for j in range(PG):
                        nc.tensor.matmul(
                            logits_ps[:, t0 + j:t0 + j + 1],
                            prod[:, j, :],
                            ones_col_s,
                            start=True, stop=True)

                # ---- softmax over time (free dim) ----
                mx = attp.tile([BS, 1], F32, tag="mx")
                nc.vector.tensor_reduce(mx[:], logits_ps[:],
                                        mybir.AxisListType.X, ALU.max)
                negmx = attp.tile([BS, 1], F32, tag="negmx")
                nc.vector.tensor_scalar_mul(negmx[:], mx[:], -1.0)
                exps = attp.tile([BS, t_steps], F32, tag="exps")
                nc.scalar.activation(exps[:], logits_ps[:], AF.Exp,
                                     bias=negmx[:])
                # swap in the sigmoid/tanh table for the scan
                nc.scalar.activation(dummy[:], dummy[:], AF.Sigmoid)
                ssum = attp.tile([BS, 1], F32, tag="ssum")
                nc.vector.tensor_reduce(ssum[:], exps[:], mybir.AxisListType.X,
                                        ALU.add)
                rsum = attp.tile([BS, 1], F32, tag="rsum")
                nc.vector.reciprocal(rsum[:], ssum[:])
                att = attp.tile([BS, t_steps], BF16, tag="attn")
                nc.vector.tensor_scalar_mul(att[:], exps[:], rsum[:])
                # transpose att -> attT rows (PE transpose, two halves)
                for hf in range(2):
                    tps = awps.tile([100, BS], BF16, tag="aw")
                    nc.tensor.transpose(tps[:], att[:, hf * 100:(hf + 1) * 100],
                                        ident_s)
                    nc.vector.tensor_copy(attT[:, hf, :], tps[:])

            # ================= phase 2: the scan =================
            with (
                tc.tile_pool(name="rmps", bufs=3, space="PSUM") as rmps,
                tc.tile_pool(name="uqps", bufs=3, space="PSUM") as uqps,
                tc.tile_pool(name="abps", bufs=2, space="PSUM") as abps,
            ):
                h_t = statep.tile([H, BS], BF16, tag="h")
                nc.vector.memset(h_t[:], 0.0)

                rm_tiles = {}
                uq_tiles = {}
                abc_tiles = {}
                row4_tiles = {}
                s1_prev = None
                t2_prev = None

                def row4_fill(c):
                    t0 = c * 4
                    row4 = scanp.tile([1, 4, BS], BF16, tag="arow")
                    nc.sync.dma_start(
                        row4[:], attT[t0 % 100:t0 % 100 + 4, t0 // 100, :])
                    row4_tiles[c] = row4

                def abc_fill(c):
                    row4 = row4_tiles.pop(c)
                    ab = abps.tile([128, 4, BS], F32, tag="abc")
                    nc.tensor.matmul(
                        ab[:].rearrange("p t b -> p (t b)"),
                        ones_row_s,
                        row4[:].rearrange("p t b -> p (t b)"),
                        start=True, stop=True)
                    abc_tiles[c] = ab

                def x_fill(t):
                    # x-side projections + bg bias for step t
                    # rm bank: [r | m],  uq bank: [u | q]
                    ht = hist_all[:, t, :]
                    rmt = rmps.tile([H, 2, BS], F32, tag="rm")
                    uqt = uqps.tile([H, 2, BS], F32, tag="uq")
                    nc.tensor.matmul(rmt[:, 0, :], WxrT_s, ht,
                                     start=True, stop=False)
                    # m-region = bg (outer product), h-matmul accumulates later
                    nc.tensor.matmul(rmt[:, 1, :], bg_row_s, ones_row_s[:, :BS],
                                     start=False, stop=False,
                                     skip_group_check=True)
                    nc.tensor.matmul(uqt[:, 0, :], WxuT_s, ht,
                                     start=True, stop=False)
                    nc.tensor.matmul(uqt[:, 1, :], WxgT_s, ht,
                                     start=False, stop=False,
                                     skip_group_check=True)
                    rm_tiles[t] = rmt
                    uq_tiles[t] = uqt

                def consume(t, h_cur):
                    nonlocal s1_prev, t2_prev
                    rmt = rm_tiles.pop(t)
                    uqt = uq_tiles.pop(t)
                    ab = abc_tiles[t // 4]
                    if t > 0:
                        # r-gate: Whr.h = Whr.t2 + Whr.s1 (t2-stream lands
                        # during the previous tanh; only s1 is serial)
                        nc.tensor.matmul(rmt[:, 0, :], WhrT_s, t2_prev[:],
                                         start=False, stop=False,
                                         skip_group_check=True)
                        nc.tensor.matmul(rmt[:, 0, :], WhrT_s, s1_prev[:],
                                         start=False, stop=True,
                                         skip_group_check=True)
                        nc.tensor.matmul(rmt[:, 1, :], WhgT_s, h_cur[:],
                                         start=False, stop=True,
                                         skip_group_check=True)
                        nc.tensor.matmul(uqt[:, 0, :], WhuT_s, h_cur[:],
                                         start=False, stop=True,
                                         skip_group_check=True)
                    r = scanp.tile([H, BS], BF16, tag="r")
                    nc.scalar.activation(r[:], rmt[:, 0, :], AF.Sigmoid,
                                         bias=br_s)
                    u = scanp.tile([H, BS], BF16, tag="u")
                    nc.scalar.activation(u[:], uqt[:, 0, :], AF.Sigmoid,
                                         bias=bu_s)
                    # rm = (mg + bg) * r     (bg pre-accumulated in PSUM)
                    rm = scanp.tile([H, BS], BF16, tag="rmv")
                    nc.vector.tensor_tensor(rm[:], rmt[:, 1, :], r[:], ALU.mult)
                    # gpre = (xq + bq) + rm
                    gpre = scanp.tile([H, BS], BF16, tag="gpre")
                    bi_gpre = nc.vector.scalar_tensor_tensor(
                        gpre[:], uqt[:, 1, :], bq_s, rm[:], ALU.add, ALU.add)
                    g_ = scanp.tile([H, BS], BF16, tag="g")
                    nc.scalar.activation(g_[:], gpre[:], AF.Tanh)
                    # v = a_t * u ; um = 1 - v ; t2 = um * h   (off-path,
                    # run during tanh -- keep them behind gpre on the DVE)
                    v = scanp.tile([H, BS], BF16, tag="v")
                    bi_v = nc.vector.tensor_tensor(v[:], u[:], ab[:, t % 4, :],
                                                   ALU.mult)
                    add_dep_helper(bi_v.ins, bi_gpre.ins, sync=False,
                                   reason="scan: v after gpre (DVE order)")
                    um = scanp.tile([H, BS], BF16, tag="um")
                    nc.vector.tensor_scalar(um[:], v[:], -1.0, 1.0,
                                            ALU.mult, ALU.add)
                    t2 = scanp.tile([H, BS], BF16, tag="t2")
                    nc.vector.tensor_tensor(t2[:], um[:], h_cur[:], ALU.mult)
                    # s1 = g * v  (serial), then h' = s1 + t2 (off-path)
                    s1 = scanp.tile([H, BS], BF16, tag="s1")
                    nc.vector.tensor_tensor(s1[:], g_[:], v[:], ALU.mult)
                    h_new = statep.tile([H, BS], BF16, tag="h")
                    nc.vector.tensor_tensor(h_new[:], s1[:], t2[:], ALU.add)
                    s1_prev, t2_prev = s1, t2
                    return h_new

                row4_fill(0)
                row4_fill(1)
                abc_fill(0)
                for t in range(-XLA, t_steps):
                    tf = t + XLA
                    if tf < t_steps:
                        if tf % 4 == 0 and tf > 0:
                            c = tf // 4
                            if c + 1 < NC4:
                                row4_fill(c + 1)
                            abc_fill(c)
                        x_fill(tf)
                    if t >= 0:
                        h_t = consume(t, h_t)

                # ---- phase 3: out = [h, targets[:,0]] @ ln2_w.T + ln2_b ----
                ops = rmps.tile([BS, H], F32, tag="rm")
                nc.tensor.matmul(ops[:], ones_row_s[:, :BS], ln2b_s,
                                 start=True, stop=False)
                nc.tensor.matmul(ops[:], h_t[:], ln2wh_s,
                                 start=False, stop=False, skip_group_check=True)
                nc.tensor.matmul(ops[:], tgt_all[:, 0, :], ln2wt_s,
                                 start=False, stop=True, skip_group_check=True)
                out_s = scanp.tile([BS, H], F32, tag="out_s")
                nc.vector.tensor_copy(out_s[:], ops[:])
                nc.sync.dma_start(out_d[:, :], out_s[:])

    nc.compile()
    return nc


def make_weight_feeds(inputs, t_steps=T):
    f32 = np.float32
    bf16 = ml_dtypes.bfloat16

    def tb(x):  # transpose to [in, out], fp32 -> bf16
        return np.ascontiguousarray(np.asarray(x, dtype=f32).T).astype(bf16)

    ln2_w = np.asarray(inputs["ln2_w"], f32)
    wblocks = [
        tb(inputs["W_w"]), tb(inputs["hu_w"]), tb(inputs["hr_w"]),
        tb(inputs["hg_w"]), tb(inputs["xu_w"]), tb(inputs["xr_w"]),
        tb(inputs["xg_w"]),
        np.ascontiguousarray(ln2_w[:, :H].T).astype(bf16),
        np.ascontiguousarray(ln2_w[:, H:].T).astype(bf16),
        np.eye(128, dtype=f32).astype(bf16),
        np.ones((128, 128), dtype=f32).astype(bf16),
    ]
    bcols = np.stack([
        np.asarray(inputs["W_b"], f32),
        np.asarray(inputs["xu_b"], f32) + np.asarray(inputs["hu_b"], f32),
        np.asarray(inputs["xr_b"], f32) + np.asarray(inputs["hr_b"], f32),
        np.asarray(inputs["xg_b"], f32),
    ], axis=1)
    brows = np.zeros((1, 2 * 128), f32)
    brows[0, :H] = np.asarray(inputs["ln2_b"], f32)
    brows[0, 128:128 + H] = np.asarray(inputs["hg_b"], f32)
    return {
        "wcat": np.ascontiguousarray(np.concatenate(wblocks, axis=1)),
        "bcols": np.ascontiguousarray(bcols),
        "brows": brows.astype(bf16),
    }


def make_core_feeds(inputs, core, t_steps=T):
    bf16 = ml_dtypes.bfloat16
    sl = slice(core * BS, (core + 1) * BS)
    tgt = np.asarray(inputs["targets"])[sl, :t_steps]
    hist = np.asarray(inputs["history_states"])[sl, :t_steps]
    return {
        # [BS, T, F] -> [F, T, BS]
        "tgtT": np.ascontiguousarray(tgt.transpose(2, 1, 0)).astype(bf16),
        "histT": np.ascontiguousarray(hist.transpose(2, 1, 0)).astype(bf16),
    }


_nc_cache = {}


def _get_nc(t_steps=T):
    if t_steps not in _nc_cache:
        _nc_cache[t_steps] = build_nc(t_steps)
    return _nc_cache[t_steps]


def kernel(**inputs):
    nc = _get_nc(T)
    wf = make_weight_feeds(inputs)
    in_maps = [{**make_core_feeds(inputs, c), **wf} for c in range(NCORES)]
    res = run_bass_kernel_spmd(nc, in_maps, list(range(NCORES)))
    out = np.concatenate([res.results[c]["out"] for c in range(NCORES)], axis=0)
    return out.astype(np.float32)
